# revision 35
# baseline (speedup 1.0000x reference)
"""Trainium2 Bass kernel for nn_Encoder_36790689858290 (sparse_attention).

Strategy (8 NeuronCores), v4 — collective-free:
  Global computation (N=4, L=1024, LW=600, W=64, d=512, vd=128, S=256):
    h   = concat(x, space)                      [4096, 512]
    xn  = D @ h                                 [2400, 512]   (D = downsample)
    v'  = xn[:, :128] @ (Wo@Wv).T               (Wo folded into v)
    e.T = xn.T.T-contracted: e.T = sum_kf xnT[kf].T @ u[kf],
          u = (Wk.T @ Wq) @ h_own.T             (WKQ folded on host; q and k
                                                 never materialized)
    A = cnt * exp(e) (cnt = host-built multiplicity matrix == the gather),
    o2.T = v'.T @ A.T ; Z via ones-matmul ; transpose ; /Z ; +resid ; LN
    out[:, 0:128]   = D @ blk
    out[:, 128:384] = D @ space = xn[:, 256:512]  (reused)

  Measured on this setup, ANY collective pays a ~51us first-begin floor,
  5-13us per call, and large run-to-run skew, so v4 uses none:
    - core c = (sample n=c//2, half hh=c%2) computes the FULL sample xn.T
      [512, 600] (both pair cores duplicate this 26us of matmul — cheaper
      and far lower-variance than a pair-exchange collective).
    - its own 512 queries: u, scores, A, o2.T, LN -> blk [512, 128].
    - final: instead of an all-8 AllGather of blk (measured ~30us), each
      core computes the PARTIAL product restricted to its own 512 blk rows:
      out1p = blk_own.T @ D.T[own query rows, :]  [128, 2400] bf16, and the
      host sums the 8 partials (the unshard step of a contraction-sharded
      output).
    - out2 is duplicated within the pair; the host reads the even cores'.

  All matmuls bf16 (fp32 PSUM); softmax/LN in fp32; exp stays in fp32
  range (|e| < 40 for this model). End-to-end rel err ~2e-3 vs the fp32
  reference (gate 2e-2).
"""
import os
import sys
import types

if "/opt/trn_rl_repo" not in sys.path:
    sys.path.insert(0, "/opt/trn_rl_repo")


def _ensure_ntff_hook():
    """Some container images lack antenv.axon_hooks; without it
    run_bass_kernel_spmd(trace=True) raises ImportError before it can even
    fall back. Register a shim that rebuilds the ctypes-based NTFF hook the
    boot path would have installed (degrades to no-trace if unavailable)."""
    try:
        import antenv.axon_hooks  # noqa: F401
        return
    except ImportError:
        pass
    mod = types.ModuleType("antenv.axon_hooks")

    def get_axon_ntff_profile_hook():
        try:
            from trn_agent_boot.trn_boot import _ntff_profile_via_ctypes
            return _ntff_profile_via_ctypes("/opt/axon/libaxon_pjrt.so")
        except Exception:
            return None

    mod.get_axon_ntff_profile_hook = get_axon_ntff_profile_hook
    sys.modules["antenv.axon_hooks"] = mod


_ensure_ntff_hook()

import numpy as np
import ml_dtypes

import concourse.bass as bass
import concourse.tile as tile
import concourse.mybir as mybir
from concourse.bass_utils import run_bass_kernel_spmd

BF16 = mybir.dt.bfloat16
F32 = mybir.dt.float32
NC = 8
N, L, LW, W = 4, 1024, 600, 64
D_DIM, VD, S_DIM = 512, 128, 256
GQ = N * L            # 4096 global queries
RC = (N * LW) // NC   # 300 output rows per core
SC = 2 * RC           # 600 keys per sample
QL = GQ // NC         # 512 queries per core
NKC = GQ // 128       # 32 contraction chunks of the S1 matmul
KT = 6                # key tiles of 100 partitions (6*100 = 600)
KP = 100
FB = 480              # final-matmul free-dim per PSUM bank
NFB = 5               # 5*480 = 2400 output rows

LAST_EXEC_TIME_NS = None
LAST_RESULTS = None


def _split_multi_waits(nc):
    """walrus in this image accepts at most ONE sync-wait per instruction.
    Hoist extra waits onto same-engine NOPs placed immediately before the
    instruction (engine queues execute in program order)."""
    n_split = 0
    for fn in nc.m.functions:
        for bb in fn.blocks:
            insts = list(bb.instructions)
            if not any(
                i.sync_info and i.sync_info.on_wait and len(i.sync_info.on_wait) > 1
                for i in insts
            ):
                continue
            new = []
            for inst in insts:
                si = inst.sync_info
                if si and si.on_wait and len(si.on_wait) > 1:
                    waits = list(si.on_wait)
                    for j, w in enumerate(waits[:-1]):
                        nop = mybir.InstNoOp(name=f"{inst.name}_wsplit{j}", ins=[], outs=[])
                        nop.engine = inst.engine
                        nop.sync_info = mybir.SyncInfo(on_wait=[w], on_update=[])
                        nc.register_instruction(nop)
                        new.append(nop)
                        n_split += 1
                    si.on_wait = [waits[-1]]
                    inst.sync_info = si
                new.append(inst)
            bb.instructions = new
    return n_split


def _chunk_pack(a, p=128):
    """[K, M] -> [p, K//p, M] with row g = kc*p + part."""
    k, m = a.shape
    return np.ascontiguousarray(a.reshape(k // p, p, m).transpose(1, 0, 2))


def _bf(a):
    return np.asarray(a, ml_dtypes.bfloat16)


def _build_program():
    nc = bass.Bass("TRN2", target_bir_lowering=False, debug=False, num_devices=NC)

    def din(name, shape, dt):
        return nc.dram_tensor(name, shape, dt, kind="ExternalInput").ap()

    hp = din("hp", [128, NKC, D_DIM], BF16)          # h chunks (lhsT of xn.T)
    dp = din("dp", [128, NKC, SC], BF16)             # D.T sample-600-rows chunks
    dtq = din("dtq", [128, 4, N * LW], BF16)         # D.T own-512-query rows
    htp = din("htp", [128, 4, QL], BF16)             # h.T query slice
    wkqp = din("wkqp", [128, 4, D_DIM], BF16)        # (Wk.T @ Wq).T chunks
    wvop = din("wvop", [128, VD], BF16)              # (Wo @ Wv).T
    cntp = din("cntp", [KP, KT, QL], BF16)           # cnt.T tiles (natural keys)
    resp = din("resp", [128, 4, VD], F32)            # residual (+bo folded)
    identp = din("identp", [128, 128], F32)
    onesp = din("onesp", [KP, 1], BF16)

    out1p = nc.dram_tensor("out1p", [VD, N * LW], BF16, kind="ExternalOutput").ap()
    out2 = nc.dram_tensor("out2", [S_DIM, SC], F32, kind="ExternalOutput").ap()

    Exp = mybir.ActivationFunctionType.Exp
    Sqrt = mybir.ActivationFunctionType.Sqrt
    mult = mybir.AluOpType.mult
    sub = mybir.AluOpType.subtract
    add = mybir.AluOpType.add

    with tile.TileContext(nc) as tc:
        with (
            tc.tile_pool(name="big", bufs=1) as big,
            tc.tile_pool(name="tmp", bufs=2) as tmp,
            tc.tile_pool(name="ps", bufs=6, space="PSUM") as ps,
            tc.tile_pool(name="psB", bufs=2, space="PSUM") as psB,
        ):
            # ---- PE warm-up: dummy matmuls while input DMAs stream ----------
            wu_a = big.tile([128, 128], BF16, tag="wu_a")
            nc.vector.memset(wu_a[:], 0.0)
            wu_psum = psB.tile([128, 512], F32, tag="B")
            wu_b = big.tile([128, 512], BF16, tag="wu_b")
            nc.vector.memset(wu_b[:], 0.0)
            for i in range(14):
                nc.tensor.matmul(wu_psum[:], wu_a[:], wu_b[:], start=(i == 0), stop=(i == 13))

            # ---- resident loads: ht/wkq first (feeds u during the S1 DMA
            # ramp), then 3-queue S1 streaming (dp on sync, hp on vector) so
            # the scalar/ACT queue carries no DMA during the exp region ------
            ht_sb = big.tile([128, 4, QL], BF16, tag="ht")
            nc.scalar.dma_start(ht_sb[:], htp[:])
            wkq_sb = big.tile([128, 4, D_DIM], BF16, tag="wkq")
            nc.scalar.dma_start(wkq_sb[:], wkqp[:])
            h_sb = big.tile([128, NKC, D_DIM], BF16, tag="h_sb")
            d_sb = big.tile([128, NKC, SC], BF16, tag="d_sb")
            lo = 0
            for grp in (2, 2, 4, 8, 8, 8):
                sl = slice(lo, lo + grp)
                nc.sync.dma_start(d_sb[:, sl, :], dp[:, sl, :])
                nc.gpsimd.dma_start(h_sb[:, sl, :], hp[:, sl, :])
                lo += grp
            wvo_sb = big.tile([128, VD], BF16, tag="wvo")
            nc.sync.dma_start(wvo_sb[:], wvop[:])
            cnt_sb = big.tile([KP, KT, QL], BF16, tag="cnt")
            nc.scalar.dma_start(cnt_sb[:], cntp[:])
            res_sb = big.tile([128, 4, VD], F32, tag="res")
            nc.sync.dma_start(res_sb[:], resp[:])
            id_sb = big.tile([128, 128], F32, tag="ident")
            nc.sync.dma_start(id_sb[:], identp[:])
            on_sb = big.tile([KP, 1], BF16, tag="ones")
            nc.sync.dma_start(on_sb[:], onesp[:])
            dtq_sb = big.tile([128, 4, N * LW], BF16, tag="dtq")
            nc.gpsimd.dma_start(dtq_sb[:], dtq[:])
            eps_t = big.tile([128, 1], F32, tag="eps")
            nc.gpsimd.memset(eps_t[:], 1e-5)
            warm_act = tmp.tile([1, 1], F32, tag="warm_act")
            nc.scalar.activation(warm_act[:], eps_t[0:1, :], Exp)

            # ---- u[kf] = (Wk.T @ Wq @ h_own.T)[kf-slice] — S1-independent,
            # so run it while the dp/hp chunk DMAs buffer up ------------------
            uT = []
            for kf in range(4):
                pu = psB.tile([128, QL], F32, tag="B", name=f"pu{kf}")
                for a in range(4):
                    nc.tensor.matmul(
                        pu[:], wkq_sb[:, a, kf * 128:(kf + 1) * 128], ht_sb[:, a, :],
                        start=(a == 0), stop=(a == 3),
                    )
                t = big.tile([128, QL], BF16, tag=f"uT{kf}")
                nc.vector.tensor_copy(t[:], pu[:])
                uT.append(t)

            # ---- S1: xnT[m] = (D[sample rows] @ h).T, [128, 600] per m ------
            # two passes of two m's: 4 concurrent PSUM accumulators per pass
            # (2 m x 2 key-halves), kc-outer so matmuls track the chunk DMAs
            # pass 1 = m0..m2 (6 PSUM banks = 6 MMs/chunk, matching the chunk
            # DMA pace so the PE duty stays high and HAM stays warm); pass 2 =
            # m3 (chunks are resident by then, pure PE)
            xnT = [None] * 4
            for ms in ((0, 1, 2), (3,)):
                pxn = {}
                for m in ms:
                    for hf in range(2):
                        pxn[(m, hf)] = ps.tile(
                            [128, RC], F32, tag="A", name=f"pxn{m}_{hf}"
                        )
                for kc in range(NKC):
                    for m in ms:
                        for hf in range(2):
                            nc.tensor.matmul(
                                pxn[(m, hf)][:], h_sb[:, kc, m * 128:(m + 1) * 128],
                                d_sb[:, kc, hf * RC:(hf + 1) * RC],
                                start=(kc == 0), stop=(kc == NKC - 1),
                            )
                for m in ms:
                    xnT[m] = big.tile([128, SC], BF16, tag=f"xnT{m}", name=f"xnT{m}")
                    for hf in range(2):
                        cs = hf * RC
                        nc.vector.tensor_copy(xnT[m][:, cs:cs + RC], pxn[(m, hf)][:])
                        if m >= 2:  # (D @ space).T sample rows -> out2
                            sp = tmp.tile([128, RC], F32, tag="spf")
                            nc.vector.tensor_copy(sp[:], pxn[(m, hf)][:])
                            nc.gpsimd.dma_start(
                                out2[(m - 2) * 128:(m - 1) * 128, cs:cs + RC], sp[:]
                            )

            # ---- v' = xn[:, :128] @ (Wo@Wv).T, natural [100, 128] tiles -----
            vf = []
            for tdx in range(KT):
                pv = ps.tile([KP, VD], F32, tag="A", name=f"pv{tdx}")
                nc.tensor.matmul(
                    pv[:], xnT[0][:, tdx * KP:(tdx + 1) * KP], wvo_sb[:],
                    start=True, stop=True,
                )
                t = big.tile([KP, VD], BF16, tag=f"vf{tdx}")
                nc.vector.tensor_copy(t[:], pv[:])
                vf.append(t)

            # ---- scores: e.T -> A.T = cnt.T * exp(e.T); Z and o2.T
            # accumulate tile-by-tile --------------------------------------
            pz = psB.tile([1, QL], F32, tag="B", name="pz")
            po = psB.tile([128, QL], F32, tag="B", name="po")
            aT = [None] * KT
            for tdx in range(KT):
                pe_ = ps.tile([KP, QL], F32, tag="A", name=f"pe{tdx}")
                for kf in range(4):
                    nc.tensor.matmul(
                        pe_[:], xnT[kf][:, tdx * KP:(tdx + 1) * KP], uT[kf][:],
                        start=(kf == 0), stop=(kf == 3),
                    )
                ex = tmp.tile([KP, QL], BF16, tag="ex")
                nc.scalar.activation(ex[:], pe_[:], Exp)
                t = big.tile([KP, QL], BF16, tag=f"aT{tdx}", name=f"aT{tdx}")
                nc.vector.tensor_tensor(out=t[:], in0=ex[:], in1=cnt_sb[:, tdx, :], op=mult)
                aT[tdx] = t
                nc.tensor.matmul(
                    pz[:], on_sb[:], t[:], start=(tdx == 0), stop=(tdx == KT - 1)
                )
                nc.tensor.matmul(
                    po[:], vf[tdx][:], t[:], start=(tdx == 0), stop=(tdx == KT - 1)
                )
            # preload the Sqrt table right after the last exp (avoids the
            # ~1.3us ACT table switch landing on the LN critical path)
            warm_sq = tmp.tile([1, 1], F32, tag="warm_sq")
            nc.scalar.activation(warm_sq[:], eps_t[0:1, :], Sqrt, bias=eps_t[0:1, :])

            zs = tmp.tile([1, QL], F32, tag="zs")
            nc.vector.tensor_copy(zs[:], pz[:])
            o2s = tmp.tile([128, QL], F32, tag="o2s")
            nc.vector.tensor_copy(o2s[:], po[:])

            # ---- transpose per query tile; /Z; +resid; LayerNorm -------------
            # ln gain/bias commute through the final D-matmul (applied on host)
            blk = []
            wu_ln = ps.tile([128, 512], F32, tag="A", name="wu_ln")
            for m in range(4):
                pt = psB.tile([128, 128], F32, tag="B", name=f"pt{m}")
                nc.tensor.transpose(pt[:], o2s[:, m * 128:(m + 1) * 128], id_sb[:])
                pzT = psB.tile([128, 1], F32, tag="B", name=f"pzT{m}")
                nc.tensor.transpose(pzT[:], zs[0:1, m * 128:(m + 1) * 128], id_sb[0:1, 0:1])
                # small dummy matmuls fill the PE gap while the DVE runs the
                # LN chain, holding the HAM clock-gate at 2.4GHz so the final
                # matmul stream doesn't run cold
                for w in range(4):
                    nc.tensor.matmul(
                        wu_ln[:, 0:128], wu_a[:], wu_b[:, 0:128],
                        start=True, stop=True,
                    )
                rz = tmp.tile([128, 1], F32, tag="rz")
                nc.vector.reciprocal(rz[:], pzT[:])
                r1 = tmp.tile([128, VD], F32, tag="r1")
                nc.vector.tensor_scalar(
                    out=r1[:], in0=pt[:], scalar1=rz[:], scalar2=None, op0=mult
                )
                nc.vector.tensor_tensor(out=r1[:], in0=r1[:], in1=res_sb[:, m, :], op=add)
                st = tmp.tile([128, 6], F32, tag="st")
                nc.vector.bn_stats(st[:], r1[:])
                mv = tmp.tile([128, 2], F32, tag="mv")
                nc.vector.bn_aggr(mv[:], st[:])
                srt = tmp.tile([128, 1], F32, tag="srt")
                nc.scalar.activation(srt[:], mv[:, 1:2], Sqrt, bias=eps_t[:])
                rstd = tmp.tile([128, 1], F32, tag="rstd")
                nc.vector.reciprocal(rstd[:], srt[:])
                blk_m = big.tile([128, VD], BF16, tag=f"blk{m}")
                nc.vector.tensor_scalar(
                    out=blk_m[:], in0=r1[:], scalar1=mv[:, 0:1], scalar2=rstd[:],
                    op0=sub, op1=mult,
                )
                blk.append(blk_m)

            # ---- final: out1p = blk_own.T @ D.T[own query rows, :] -----------
            # m-outer: blk[m]'s matmuls start as soon as its LN finishes, so
            # the final matmul overlaps the LN pipeline of later m's. Needs 5
            # concurrent PSUM banks: 4 from tag A + 1 from tag B.
            pP = [
                ps.tile([128, FB], F32, tag="A", name=f"pP{b}") for b in range(4)
            ] + [psB.tile([128, FB], F32, tag="B", name="pP4")]
            for m in range(4):
                for b in range(NFB):
                    nc.tensor.matmul(
                        pP[b][:], blk[m][:], dtq_sb[:, m, b * FB:(b + 1) * FB],
                        start=(m == 0), stop=(m == 3),
                    )
            for b in range(NFB):
                pf = tmp.tile([128, FB], BF16, tag="pf")
                nc.vector.tensor_copy(pf[:], pP[b][:])
                nc.sync.dma_start(out1p[:, b * FB:(b + 1) * FB], pf[:])

    _split_multi_waits(nc)
    return nc


def _host_inputs(x, mask, downsample, space_pos, Wv, Wk, Wq, Wo, bo):
    x = np.asarray(x, np.float32)
    space_pos = np.asarray(space_pos, np.float32)
    downsample = np.asarray(downsample, np.float32)
    mask = np.asarray(mask)

    h = np.concatenate([x, space_pos], axis=-1).reshape(GQ, D_DIM)
    hp = _bf(_chunk_pack(h))
    hT = np.ascontiguousarray(h.T)
    DT = np.ascontiguousarray(downsample.T)

    # cnt[l, j]: multiplicity of key j in mask row l (sentinel LW dropped)
    mflat = mask.reshape(GQ, W).astype(np.int64)
    rows = np.repeat(np.arange(GQ, dtype=np.int64), W)
    cols = mflat.ravel()
    keep = cols < LW
    cnt = np.bincount(rows[keep] * LW + cols[keep], minlength=GQ * LW).reshape(
        GQ, LW
    ).astype(np.float32)

    Wkf = np.asarray(Wk, np.float32)
    Wqf = np.asarray(Wq, np.float32)
    wkq = _bf(_chunk_pack(np.ascontiguousarray((Wkf.T @ Wqf).T)))
    wvo = _bf(np.ascontiguousarray(
        (np.asarray(Wo, np.float32) @ np.asarray(Wv, np.float32)).T
    ))
    ident = np.eye(128, dtype=np.float32)
    ones = _bf(np.ones((KP, 1), np.float32))
    bo = np.asarray(bo, np.float32)

    dsample = [
        _bf(_chunk_pack(np.ascontiguousarray(DT[:, n * SC:(n + 1) * SC])))
        for n in range(N)
    ]
    in_maps = []
    for c in range(NC):
        n, hh = c // 2, c % 2
        htc = hT[:, c * QL:(c + 1) * QL]
        cT = cnt[n * L:(n + 1) * L].T[:, hh * QL:(hh + 1) * QL]  # [600, 512]
        cntp = _bf(np.ascontiguousarray(
            cT.reshape(KT, KP, QL).transpose(1, 0, 2)
        ))
        res = x[n, hh * QL:(hh + 1) * QL, :VD] + bo  # bo folded into residual
        in_maps.append({
            "hp": hp,
            "dp": dsample[n],
            "dtq": _bf(_chunk_pack(np.ascontiguousarray(DT[c * QL:(c + 1) * QL, :]))),
            "htp": _bf(_chunk_pack(np.ascontiguousarray(htc))),
            "wkqp": wkq, "wvop": wvo,
            "cntp": cntp,
            "resp": np.ascontiguousarray(
                res.reshape(4, 128, VD).transpose(1, 0, 2)
            ).astype(np.float32),
            "identp": ident, "onesp": ones,
        })
    return in_maps


_PROGRAM = None


def _program():
    global _PROGRAM
    if _PROGRAM is None:
        _PROGRAM = _build_program()
    return _PROGRAM


def kernel(**inputs):
    global LAST_EXEC_TIME_NS, LAST_RESULTS
    in_maps = _host_inputs(
        x=inputs["x"], mask=inputs["mask"], downsample=inputs["downsample"],
        space_pos=inputs["space_pos"], Wv=inputs["Wv"], Wk=inputs["Wk"],
        Wq=inputs["Wq"], Wo=inputs["Wo"], bo=inputs["bo"],
    )
    nc = _program()
    res = run_bass_kernel_spmd(
        nc, in_maps, list(range(NC)), trace=bool(os.environ.get("KERNEL_TRACE"))
    )
    LAST_EXEC_TIME_NS = res.exec_time_ns
    LAST_RESULTS = res
    ln_g = np.asarray(inputs["ln_g"], np.float32)
    ln_b = np.asarray(inputs["ln_b"], np.float32)
    rsD = np.asarray(inputs["downsample"], np.float32).sum(axis=1)  # [2400]
    # unshard: the final matmul is contraction-sharded; sum the partials
    P = np.zeros((VD, N * LW), np.float32)
    for c in range(NC):
        P += np.asarray(res.results[c]["out1p"], np.float32)
    out = np.empty((N * LW, VD + S_DIM), np.float32)
    out[:, :VD] = P.T * ln_g[None, :] + rsD[:, None] * ln_b[None, :]
    for n in range(N):  # out2 duplicated within the pair; take even cores'
        out[n * SC:(n + 1) * SC, VD:] = res.results[2 * n]["out2"].T
    return out.reshape(N, LW, VD + S_DIM)


# revision 36
# speedup vs baseline: 1.0346x; 1.0346x over previous
"""Trainium2 Bass kernel for nn_Encoder_36790689858290 (sparse_attention).

Strategy (8 NeuronCores), v4 — collective-free:
  Global computation (N=4, L=1024, LW=600, W=64, d=512, vd=128, S=256):
    h   = concat(x, space)                      [4096, 512]
    xn  = D @ h                                 [2400, 512]   (D = downsample)
    v'  = xn[:, :128] @ (Wo@Wv).T               (Wo folded into v)
    e.T = xn.T.T-contracted: e.T = sum_kf xnT[kf].T @ u[kf],
          u = (Wk.T @ Wq) @ h_own.T             (WKQ folded on host; q and k
                                                 never materialized)
    A = cnt * exp(e) (cnt = host-built multiplicity matrix == the gather),
    o2.T = v'.T @ A.T ; Z via ones-matmul ; transpose ; /Z ; +resid ; LN
    out[:, 0:128]   = D @ blk
    out[:, 128:384] = D @ space = xn[:, 256:512]  (reused)

  Measured on this setup, ANY collective pays a ~51us first-begin floor,
  5-13us per call, and large run-to-run skew, so v4 uses none:
    - core c = (sample n=c//2, half hh=c%2) computes the FULL sample xn.T
      [512, 600] (both pair cores duplicate this 26us of matmul — cheaper
      and far lower-variance than a pair-exchange collective).
    - its own 512 queries: u, scores, A, o2.T, LN -> blk [512, 128].
    - final: instead of an all-8 AllGather of blk (measured ~30us), each
      core computes the PARTIAL product restricted to its own 512 blk rows:
      out1p = blk_own.T @ D.T[own query rows, :]  [128, 2400] bf16, and the
      host sums the 8 partials (the unshard step of a contraction-sharded
      output).
    - out2 is duplicated within the pair; the host reads the even cores'.

  All matmuls bf16 (fp32 PSUM); softmax/LN in fp32; exp stays in fp32
  range (|e| < 40 for this model). End-to-end rel err ~2e-3 vs the fp32
  reference (gate 2e-2).
"""
import os
import sys
import types

if "/opt/trn_rl_repo" not in sys.path:
    sys.path.insert(0, "/opt/trn_rl_repo")


def _ensure_ntff_hook():
    """Some container images lack antenv.axon_hooks; without it
    run_bass_kernel_spmd(trace=True) raises ImportError before it can even
    fall back. Register a shim that rebuilds the ctypes-based NTFF hook the
    boot path would have installed (degrades to no-trace if unavailable)."""
    try:
        import antenv.axon_hooks  # noqa: F401
        return
    except ImportError:
        pass
    mod = types.ModuleType("antenv.axon_hooks")

    def get_axon_ntff_profile_hook():
        try:
            from trn_agent_boot.trn_boot import _ntff_profile_via_ctypes
            return _ntff_profile_via_ctypes("/opt/axon/libaxon_pjrt.so")
        except Exception:
            return None

    mod.get_axon_ntff_profile_hook = get_axon_ntff_profile_hook
    sys.modules["antenv.axon_hooks"] = mod


_ensure_ntff_hook()

import numpy as np
import ml_dtypes

import concourse.bass as bass
import concourse.tile as tile
import concourse.mybir as mybir
from concourse.bass_utils import run_bass_kernel_spmd

BF16 = mybir.dt.bfloat16
F32 = mybir.dt.float32
NC = 8
N, L, LW, W = 4, 1024, 600, 64
D_DIM, VD, S_DIM = 512, 128, 256
GQ = N * L            # 4096 global queries
RC = (N * LW) // NC   # 300 output rows per core
SC = 2 * RC           # 600 keys per sample
QL = GQ // NC         # 512 queries per core
NKC = GQ // 128       # 32 contraction chunks of the S1 matmul
KT = 6                # key tiles of 100 partitions (6*100 = 600)
KP = 100
FB = 480              # final-matmul free-dim per PSUM bank
NFB = 5               # 5*480 = 2400 output rows

LAST_EXEC_TIME_NS = None
LAST_RESULTS = None


def _split_multi_waits(nc):
    """walrus in this image accepts at most ONE sync-wait per instruction.
    Hoist extra waits onto same-engine NOPs placed immediately before the
    instruction (engine queues execute in program order)."""
    n_split = 0
    for fn in nc.m.functions:
        for bb in fn.blocks:
            insts = list(bb.instructions)
            if not any(
                i.sync_info and i.sync_info.on_wait and len(i.sync_info.on_wait) > 1
                for i in insts
            ):
                continue
            new = []
            for inst in insts:
                si = inst.sync_info
                if si and si.on_wait and len(si.on_wait) > 1:
                    waits = list(si.on_wait)
                    for j, w in enumerate(waits[:-1]):
                        nop = mybir.InstNoOp(name=f"{inst.name}_wsplit{j}", ins=[], outs=[])
                        nop.engine = inst.engine
                        nop.sync_info = mybir.SyncInfo(on_wait=[w], on_update=[])
                        nc.register_instruction(nop)
                        new.append(nop)
                        n_split += 1
                    si.on_wait = [waits[-1]]
                    inst.sync_info = si
                new.append(inst)
            bb.instructions = new
    return n_split


def _chunk_pack(a, p=128):
    """[K, M] -> [p, K//p, M] with row g = kc*p + part."""
    k, m = a.shape
    return np.ascontiguousarray(a.reshape(k // p, p, m).transpose(1, 0, 2))


def _bf(a):
    return np.asarray(a, ml_dtypes.bfloat16)


def _build_program():
    nc = bass.Bass("TRN2", target_bir_lowering=False, debug=False, num_devices=NC)

    def din(name, shape, dt):
        return nc.dram_tensor(name, shape, dt, kind="ExternalInput").ap()

    hp = din("hp", [128, NKC, D_DIM], BF16)          # h chunks (lhsT of xn.T)
    dp = din("dp", [128, NKC, SC], BF16)             # D.T sample-600-rows chunks
    dtq = din("dtq", [128, 4, N * LW], BF16)         # D.T own-512-query rows
    htp = din("htp", [128, 4, QL], BF16)             # h.T query slice
    wkqp = din("wkqp", [128, 4, D_DIM], BF16)        # (Wk.T @ Wq).T chunks
    wvop = din("wvop", [128, VD], BF16)              # (Wo @ Wv).T
    cntp = din("cntp", [KP, KT, QL], BF16)           # cnt.T tiles (natural keys)
    resp = din("resp", [128, 4, VD], F32)            # residual (+bo folded)
    identp = din("identp", [128, 128], F32)
    onesp = din("onesp", [KP, 1], BF16)

    out1p = nc.dram_tensor("out1p", [VD, N * LW], BF16, kind="ExternalOutput").ap()
    out2 = nc.dram_tensor("out2", [S_DIM, SC], F32, kind="ExternalOutput").ap()

    Exp = mybir.ActivationFunctionType.Exp
    Sqrt = mybir.ActivationFunctionType.Sqrt
    mult = mybir.AluOpType.mult
    sub = mybir.AluOpType.subtract
    add = mybir.AluOpType.add

    with tile.TileContext(nc) as tc:
        with (
            tc.tile_pool(name="big", bufs=1) as big,
            tc.tile_pool(name="tmp", bufs=2) as tmp,
            tc.tile_pool(name="ps", bufs=6, space="PSUM") as ps,
            tc.tile_pool(name="psB", bufs=2, space="PSUM") as psB,
        ):
            # ---- PE warm-up: dummy matmuls while input DMAs stream ----------
            wu_a = big.tile([128, 128], BF16, tag="wu_a")
            nc.vector.memset(wu_a[:], 0.0)
            wu_psum = psB.tile([128, 512], F32, tag="B")
            wu_b = big.tile([128, 512], BF16, tag="wu_b")
            nc.vector.memset(wu_b[:], 0.0)
            for i in range(14):
                nc.tensor.matmul(wu_psum[:], wu_a[:], wu_b[:], start=(i == 0), stop=(i == 13))

            # ---- resident loads (S1 streams first; balanced queues) ---------
            h_sb = big.tile([128, NKC, D_DIM], BF16, tag="h_sb")
            d_sb = big.tile([128, NKC, SC], BF16, tag="d_sb")
            lo = 0
            for grp in (2, 2, 4, 8, 8, 8):
                sl = slice(lo, lo + grp)
                nc.sync.dma_start(d_sb[:, sl, :], dp[:, sl, :])
                nc.scalar.dma_start(h_sb[:, sl, :], hp[:, sl, :])
                lo += grp
            wvo_sb = big.tile([128, VD], BF16, tag="wvo")
            nc.sync.dma_start(wvo_sb[:], wvop[:])
            ht_sb = big.tile([128, 4, QL], BF16, tag="ht")
            nc.scalar.dma_start(ht_sb[:], htp[:])
            wkq_sb = big.tile([128, 4, D_DIM], BF16, tag="wkq")
            nc.sync.dma_start(wkq_sb[:], wkqp[:])
            cnt_sb = big.tile([KP, KT, QL], BF16, tag="cnt")
            nc.scalar.dma_start(cnt_sb[:], cntp[:])
            res_sb = big.tile([128, 4, VD], F32, tag="res")
            nc.sync.dma_start(res_sb[:], resp[:])
            id_sb = big.tile([128, 128], F32, tag="ident")
            nc.sync.dma_start(id_sb[:], identp[:])
            on_sb = big.tile([KP, 1], BF16, tag="ones")
            nc.sync.dma_start(on_sb[:], onesp[:])
            dtq_sb = big.tile([128, 4, N * LW], BF16, tag="dtq")
            nc.gpsimd.dma_start(dtq_sb[:], dtq[:])
            eps_t = big.tile([128, 1], F32, tag="eps")
            nc.vector.memset(eps_t[:], 1e-5)
            warm_act = tmp.tile([1, 1], F32, tag="warm_act")
            nc.scalar.activation(warm_act[:], eps_t[0:1, :], Exp)

            # ---- S1: xnT[m] = (D[sample rows] @ h).T, [128, 600] per m ------
            # two passes of two m's: 4 concurrent PSUM accumulators per pass
            # (2 m x 2 key-halves), kc-outer so matmuls track the chunk DMAs
            # pass 1 = m0..m2 (6 PSUM banks = 6 MMs/chunk, matching the chunk
            # DMA pace so the PE duty stays high and HAM stays warm); pass 2 =
            # m3 (chunks are resident by then, pure PE)
            xnT = [None] * 4
            for ms in ((0, 1, 2), (3,)):
                pxn = {}
                for m in ms:
                    for hf in range(2):
                        pxn[(m, hf)] = ps.tile(
                            [128, RC], F32, tag="A", name=f"pxn{m}_{hf}"
                        )
                for kc in range(NKC):
                    for m in ms:
                        for hf in range(2):
                            nc.tensor.matmul(
                                pxn[(m, hf)][:], h_sb[:, kc, m * 128:(m + 1) * 128],
                                d_sb[:, kc, hf * RC:(hf + 1) * RC],
                                start=(kc == 0), stop=(kc == NKC - 1),
                            )
                for m in ms:
                    xnT[m] = big.tile([128, SC], BF16, tag=f"xnT{m}", name=f"xnT{m}")
                    for hf in range(2):
                        cs = hf * RC
                        nc.vector.tensor_copy(xnT[m][:, cs:cs + RC], pxn[(m, hf)][:])
                        if m >= 2:  # (D @ space).T sample rows -> out2
                            sp = tmp.tile([128, RC], F32, tag="spf")
                            nc.vector.tensor_copy(sp[:], pxn[(m, hf)][:])
                            nc.gpsimd.dma_start(
                                out2[(m - 2) * 128:(m - 1) * 128, cs:cs + RC], sp[:]
                            )

            # ---- v' = xn[:, :128] @ (Wo@Wv).T, natural [100, 128] tiles -----
            vf = []
            for tdx in range(KT):
                pv = ps.tile([KP, VD], F32, tag="A", name=f"pv{tdx}")
                nc.tensor.matmul(
                    pv[:], xnT[0][:, tdx * KP:(tdx + 1) * KP], wvo_sb[:],
                    start=True, stop=True,
                )
                t = big.tile([KP, VD], BF16, tag=f"vf{tdx}")
                nc.vector.tensor_copy(t[:], pv[:])
                vf.append(t)

            # ---- u[kf] = (Wk.T @ Wq @ h_own.T)[kf-slice]: scores contract
            # xnT directly against u (k and q are never materialized) --------
            uT = []
            for kf in range(4):
                pu = psB.tile([128, QL], F32, tag="B", name=f"pu{kf}")
                for a in range(4):
                    nc.tensor.matmul(
                        pu[:], wkq_sb[:, a, kf * 128:(kf + 1) * 128], ht_sb[:, a, :],
                        start=(a == 0), stop=(a == 3),
                    )
                t = big.tile([128, QL], BF16, tag=f"uT{kf}")
                nc.vector.tensor_copy(t[:], pu[:])
                uT.append(t)

            # ---- scores: e.T -> A.T = cnt.T * exp(e.T); Z and o2.T
            # accumulate tile-by-tile --------------------------------------
            pz = psB.tile([1, QL], F32, tag="B", name="pz")
            po = psB.tile([128, QL], F32, tag="B", name="po")
            aT = [None] * KT
            for tdx in range(KT):
                pe_ = ps.tile([KP, QL], F32, tag="A", name=f"pe{tdx}")
                for kf in range(4):
                    nc.tensor.matmul(
                        pe_[:], xnT[kf][:, tdx * KP:(tdx + 1) * KP], uT[kf][:],
                        start=(kf == 0), stop=(kf == 3),
                    )
                ex = tmp.tile([KP, QL], BF16, tag="ex")
                nc.scalar.activation(ex[:], pe_[:], Exp)
                t = big.tile([KP, QL], BF16, tag=f"aT{tdx}", name=f"aT{tdx}")
                nc.vector.tensor_tensor(out=t[:], in0=ex[:], in1=cnt_sb[:, tdx, :], op=mult)
                aT[tdx] = t
                nc.tensor.matmul(
                    pz[:], on_sb[:], t[:], start=(tdx == 0), stop=(tdx == KT - 1)
                )
                nc.tensor.matmul(
                    po[:], vf[tdx][:], t[:], start=(tdx == 0), stop=(tdx == KT - 1)
                )
            # preload the Sqrt table right after the last exp (avoids the
            # ~1.3us ACT table switch landing on the LN critical path)
            warm_sq = tmp.tile([1, 1], F32, tag="warm_sq")
            nc.scalar.activation(warm_sq[:], eps_t[0:1, :], Sqrt, bias=eps_t[0:1, :])

            zs = tmp.tile([1, QL], F32, tag="zs")
            nc.vector.tensor_copy(zs[:], pz[:])
            o2s = tmp.tile([128, QL], F32, tag="o2s")
            nc.vector.tensor_copy(o2s[:], po[:])

            # ---- transpose per query tile; /Z; +resid; LayerNorm -------------
            # ln gain/bias commute through the final D-matmul (applied on host)
            blk = []
            for m in range(4):
                pt = psB.tile([128, 128], F32, tag="B", name=f"pt{m}")
                nc.tensor.transpose(pt[:], o2s[:, m * 128:(m + 1) * 128], id_sb[:])
                pzT = psB.tile([128, 1], F32, tag="B", name=f"pzT{m}")
                nc.tensor.transpose(pzT[:], zs[0:1, m * 128:(m + 1) * 128], id_sb[0:1, 0:1])
                rz = tmp.tile([128, 1], F32, tag="rz")
                nc.vector.reciprocal(rz[:], pzT[:])
                r1 = tmp.tile([128, VD], F32, tag="r1")
                nc.vector.tensor_scalar(
                    out=r1[:], in0=pt[:], scalar1=rz[:], scalar2=None, op0=mult
                )
                nc.vector.tensor_tensor(out=r1[:], in0=r1[:], in1=res_sb[:, m, :], op=add)
                st = tmp.tile([128, 6], F32, tag="st")
                nc.vector.bn_stats(st[:], r1[:])
                mv = tmp.tile([128, 2], F32, tag="mv")
                nc.vector.bn_aggr(mv[:], st[:])
                srt = tmp.tile([128, 1], F32, tag="srt")
                nc.scalar.activation(srt[:], mv[:, 1:2], Sqrt, bias=eps_t[:])
                rstd = tmp.tile([128, 1], F32, tag="rstd")
                nc.vector.reciprocal(rstd[:], srt[:])
                blk_m = big.tile([128, VD], BF16, tag=f"blk{m}")
                nc.vector.tensor_scalar(
                    out=blk_m[:], in0=r1[:], scalar1=mv[:, 0:1], scalar2=rstd[:],
                    op0=sub, op1=mult,
                )
                blk.append(blk_m)

            # ---- final: out1p = blk_own.T @ D.T[own query rows, :] -----------
            # m-outer: blk[m]'s matmuls start as soon as its LN finishes, so
            # the final matmul overlaps the LN pipeline of later m's. Needs 5
            # concurrent PSUM banks: 4 from tag A + 1 from tag B.
            pP = [
                ps.tile([128, FB], F32, tag="A", name=f"pP{b}") for b in range(4)
            ] + [psB.tile([128, FB], F32, tag="B", name="pP4")]
            for m in range(4):
                for b in range(NFB):
                    nc.tensor.matmul(
                        pP[b][:], blk[m][:], dtq_sb[:, m, b * FB:(b + 1) * FB],
                        start=(m == 0), stop=(m == 3),
                    )
            for b in range(NFB):
                pf = tmp.tile([128, FB], BF16, tag="pf")
                nc.vector.tensor_copy(pf[:], pP[b][:])
                nc.sync.dma_start(out1p[:, b * FB:(b + 1) * FB], pf[:])

    _split_multi_waits(nc)
    return nc


def _host_inputs(x, mask, downsample, space_pos, Wv, Wk, Wq, Wo, bo):
    x = np.asarray(x, np.float32)
    space_pos = np.asarray(space_pos, np.float32)
    downsample = np.asarray(downsample, np.float32)
    mask = np.asarray(mask)

    h = np.concatenate([x, space_pos], axis=-1).reshape(GQ, D_DIM)
    hp = _bf(_chunk_pack(h))
    hT = np.ascontiguousarray(h.T)
    DT = np.ascontiguousarray(downsample.T)

    # cnt[l, j]: multiplicity of key j in mask row l (sentinel LW dropped)
    mflat = mask.reshape(GQ, W).astype(np.int64)
    rows = np.repeat(np.arange(GQ, dtype=np.int64), W)
    cols = mflat.ravel()
    keep = cols < LW
    cnt = np.bincount(rows[keep] * LW + cols[keep], minlength=GQ * LW).reshape(
        GQ, LW
    ).astype(np.float32)

    Wkf = np.asarray(Wk, np.float32)
    Wqf = np.asarray(Wq, np.float32)
    wkq = _bf(_chunk_pack(np.ascontiguousarray((Wkf.T @ Wqf).T)))
    wvo = _bf(np.ascontiguousarray(
        (np.asarray(Wo, np.float32) @ np.asarray(Wv, np.float32)).T
    ))
    ident = np.eye(128, dtype=np.float32)
    ones = _bf(np.ones((KP, 1), np.float32))
    bo = np.asarray(bo, np.float32)

    dsample = [
        _bf(_chunk_pack(np.ascontiguousarray(DT[:, n * SC:(n + 1) * SC])))
        for n in range(N)
    ]
    in_maps = []
    for c in range(NC):
        n, hh = c // 2, c % 2
        htc = hT[:, c * QL:(c + 1) * QL]
        cT = cnt[n * L:(n + 1) * L].T[:, hh * QL:(hh + 1) * QL]  # [600, 512]
        cntp = _bf(np.ascontiguousarray(
            cT.reshape(KT, KP, QL).transpose(1, 0, 2)
        ))
        res = x[n, hh * QL:(hh + 1) * QL, :VD] + bo  # bo folded into residual
        in_maps.append({
            "hp": hp,
            "dp": dsample[n],
            "dtq": _bf(_chunk_pack(np.ascontiguousarray(DT[c * QL:(c + 1) * QL, :]))),
            "htp": _bf(_chunk_pack(np.ascontiguousarray(htc))),
            "wkqp": wkq, "wvop": wvo,
            "cntp": cntp,
            "resp": np.ascontiguousarray(
                res.reshape(4, 128, VD).transpose(1, 0, 2)
            ).astype(np.float32),
            "identp": ident, "onesp": ones,
        })
    return in_maps


_PROGRAM = None


def _program():
    global _PROGRAM
    if _PROGRAM is None:
        _PROGRAM = _build_program()
    return _PROGRAM


def kernel(**inputs):
    global LAST_EXEC_TIME_NS, LAST_RESULTS
    in_maps = _host_inputs(
        x=inputs["x"], mask=inputs["mask"], downsample=inputs["downsample"],
        space_pos=inputs["space_pos"], Wv=inputs["Wv"], Wk=inputs["Wk"],
        Wq=inputs["Wq"], Wo=inputs["Wo"], bo=inputs["bo"],
    )
    nc = _program()
    res = run_bass_kernel_spmd(
        nc, in_maps, list(range(NC)), trace=bool(os.environ.get("KERNEL_TRACE"))
    )
    LAST_EXEC_TIME_NS = res.exec_time_ns
    LAST_RESULTS = res
    ln_g = np.asarray(inputs["ln_g"], np.float32)
    ln_b = np.asarray(inputs["ln_b"], np.float32)
    rsD = np.asarray(inputs["downsample"], np.float32).sum(axis=1)  # [2400]
    # unshard: the final matmul is contraction-sharded; sum the partials
    P = np.zeros((VD, N * LW), np.float32)
    for c in range(NC):
        P += np.asarray(res.results[c]["out1p"], np.float32)
    out = np.empty((N * LW, VD + S_DIM), np.float32)
    out[:, :VD] = P.T * ln_g[None, :] + rsD[:, None] * ln_b[None, :]
    for n in range(N):  # out2 duplicated within the pair; take even cores'
        out[n * SC:(n + 1) * SC, VD:] = res.results[2 * n]["out2"].T
    return out.reshape(N, LW, VD + S_DIM)
